# revision 12
# baseline (speedup 1.0000x reference)
"""Skip-gram negative-sampling loss on 8 Trainium2 NeuronCores.

Strategy (data-parallel over batch, hint-conformant):
  - Each core handles 2048 batch rows and 512 hierarchy pairs. Embedding
    tables are cast to bf16 on the host and replicated to every core as
    four 25000-row bucket slices per table (dma_gather requires offset-0
    sources and int16 indices, so the vocab is range-bucketed).
  - Every loss term is a pair of embedding rows:
      pos:  (in_embed[input_b],  out_embed[pos_label])   -> softplus(-dot)
      neg:  (in_embed[input_b],  out_embed[neg_label])   -> softplus(+dot)
      hier: (in_embed[a],        in_embed[b])            -> ||a - b||^2
    All three are order-invariant sums, so pairs are processed in a
    permuted "stream": sorted by (left-bucket, right-bucket), padded to
    multiples of 128 per group, group sizes maxed across cores so the
    SPMD program is identical on all 8 cores.
  - Both rows of each pair are gathered per-pair with InstDMAGatherAnt
    (row-major: stream position k -> partition k%128, slot k//128), so the
    two gathered tiles are pair-aligned by construction. DVE then does
    mult (or sub+square for hier slots) -> add-halves -> segmented reduce,
    giving one value per pair ("dots").
  - log_sigmoid terms: -softplus(v); softplus(v) = max(v,0)+ln(1+exp(-|v|))
    via ACT Abs/Exp/Ln (one table set); the per-region sign (pos vs neg)
    is handled by two tensor_scalar variants.
  - Output per core: [128, 2] partial sums; the host sums in f64 and
    subtracts the (host-computable) contribution of padding pairs.
"""

import numpy as np
import ml_dtypes

import concourse.bacc as bacc
import concourse.tile as tile
from concourse import mybir

# Problem shape (hardcoded per contract).
B = 16384
V = 100000
D = 128
C = 10
NEG = 50
PH = 4096
NCORES = 8
P = 128

BL = B // NCORES          # 2048 batch rows per core
HLC = PH // NCORES        # 512 hierarchy pairs per core

NBUCKET = 4
BUCKET = 25000            # rows per vocab bucket (< 32768 for int16)
CHUNK_SLOTS = 64          # stream slots (of 128 pairs) per compute chunk
HIER_COMPUTE = "full"     # debug knob: "full" (sub+square) or "mult"

BF16 = mybir.dt.bfloat16
F32 = mybir.dt.float32
I16 = mybir.dt.int16

NG = NBUCKET * NBUCKET


def _pad128(n):
    return -(-n // 128) * 128


def make_plan(input_labels, pos_labels, neg_labels, hierarchy_pairs):
    """Builds per-core index streams with a core-uniform call structure."""
    il = np.asarray(input_labels).astype(np.int64)
    pl = np.asarray(pos_labels).astype(np.int64)
    nl = np.asarray(neg_labels).astype(np.int64)
    hp = np.asarray(hierarchy_pairs).astype(np.int64)

    # kinds: 0=pos, 1=neg, 2=hier. Each pair = (left_label, right_label);
    # left gathers from win buckets; right from wout (pos/neg) or win (hier).
    per_core_groups = []  # [core][kind][g] -> (left_arr, right_arr)
    for k in range(NCORES):
        sl = slice(k * BL, (k + 1) * BL)
        hsl = slice(k * HLC, (k + 1) * HLC)
        ilk = il[sl]
        kind_pairs = [
            (np.repeat(ilk, C), pl[sl].reshape(-1)),
            (np.repeat(ilk, NEG), nl[sl].reshape(-1)),
            (hp[hsl, 0], hp[hsl, 1]),
        ]
        groups_by_kind = []
        for left, right in kind_pairs:
            gid = (left // BUCKET) * NBUCKET + (right // BUCKET)
            order = np.argsort(gid, kind="stable")
            l_s, r_s, gid_s = left[order], right[order], gid[order]
            bounds = np.searchsorted(gid_s, np.arange(NG + 1))
            groups_by_kind.append([
                (l_s[bounds[g] : bounds[g + 1]], r_s[bounds[g] : bounds[g + 1]])
                for g in range(NG)
            ])
        per_core_groups.append(groups_by_kind)

    caps = []  # [kind][g]
    for kind in range(3):
        caps.append([
            _pad128(max(
                max(len(per_core_groups[k][kind][g][0]) for k in range(NCORES)), 1
            ))
            for g in range(NG)
        ])

    s_pos = sum(caps[0]) // 128
    s_neg = sum(caps[1]) // 128
    s_hier = sum(caps[2]) // 128
    s_total = s_pos + s_neg + s_hier
    total_pairs = s_total * 128

    # Call lists (uniform across cores): left side one range per
    # (kind, left_bucket); right side one range per (kind, group).
    left_calls = []   # (lo, hi, bucket, kind)
    right_calls = []  # (lo, hi, bucket, kind)
    off = 0
    for kind in range(3):
        for lb in range(NBUCKET):
            blk_lo = off
            for rb in range(NBUCKET):
                g = lb * NBUCKET + rb
                right_calls.append((off, off + caps[kind][g], rb, kind))
                off += caps[kind][g]
            left_calls.append((blk_lo, off, lb, kind))
    assert off == total_pairs

    meta = {
        "s_pos": s_pos,
        "s_neg": s_neg,
        "s_hier": s_hier,
        "s_total": s_total,
        "left_calls": left_calls,
        "right_calls": right_calls,
    }

    per_core = []
    for k in range(NCORES):
        left_idx = np.zeros(total_pairs, np.int16)
        right_idx = np.zeros(total_pairs, np.int16)
        pad_info = []  # (count, left_bucket, right_bucket, kind)
        off = 0
        for kind in range(3):
            for lb in range(NBUCKET):
                for rb in range(NBUCKET):
                    g = lb * NBUCKET + rb
                    la, ra = per_core_groups[k][kind][g]
                    n = len(la)
                    cap = caps[kind][g]
                    left_idx[off : off + n] = la - lb * BUCKET
                    right_idx[off : off + n] = ra - rb * BUCKET
                    if cap > n:
                        pad_info.append((cap - n, lb, rb, kind))
                    off += cap
        per_core.append({
            "left_idx": np.tile(left_idx.reshape(-1, 16).T, (8, 1)).copy(),
            "right_idx": np.tile(right_idx.reshape(-1, 16).T, (8, 1)).copy(),
            "pad_info": pad_info,
        })
    return meta, per_core


def build_program(meta, bucket_rows=BUCKET, enable_asserts=False):
    s_pos, s_neg, s_hier = meta["s_pos"], meta["s_neg"], meta["s_hier"]
    s_total = meta["s_total"]
    s_pn = s_pos + s_neg
    total_pairs = s_total * 128

    nc = bacc.Bacc(
        "TRN2",
        target_bir_lowering=False,
        debug=False,
        enable_asserts=enable_asserts,
        num_devices=NCORES,
    )

    win_b = [
        nc.dram_tensor(f"win{r}", [bucket_rows, D], BF16, kind="ExternalInput").ap()
        for r in range(NBUCKET)
    ]
    wout_b = [
        nc.dram_tensor(f"wout{r}", [bucket_rows, D], BF16, kind="ExternalInput").ap()
        for r in range(NBUCKET)
    ]
    left_idx_d = nc.dram_tensor(
        "left_idx", [P, total_pairs // 16], I16, kind="ExternalInput"
    ).ap()
    right_idx_d = nc.dram_tensor(
        "right_idx", [P, total_pairs // 16], I16, kind="ExternalInput"
    ).ap()
    out_d = nc.dram_tensor("out", [P, 2], F32, kind="ExternalOutput").ap()

    with tile.TileContext(nc) as tc:
        with (
            tc.tile_pool(name="idx", bufs=1) as idxp,
            tc.tile_pool(name="gath", bufs=2) as gp,
            tc.tile_pool(name="alig", bufs=2) as ap_,
            tc.tile_pool(name="prod", bufs=2) as prodp,
            tc.tile_pool(name="s1", bufs=2) as s1p,
            tc.tile_pool(name="dots", bufs=1) as dotsp,
            tc.tile_pool(name="end", bufs=1) as endp,
        ):
            left_idx = idxp.tile([P, total_pairs // 16], I16)
            nc.sync.dma_start(left_idx[:], left_idx_d)
            right_idx = idxp.tile([P, total_pairs // 16], I16)
            nc.sync.dma_start(right_idx[:], right_idx_d)

            dots = dotsp.tile([P, s_total], F32)

            # chunk list: cover [0, s_pn) then the hier region [s_pn, s_total)
            chunks = [
                (c0, min(c0 + CHUNK_SLOTS, s_pn))
                for c0 in range(0, s_pn, CHUNK_SLOTS)
            ] + [
                (c0, min(c0 + CHUNK_SLOTS, s_total))
                for c0 in range(s_pn, s_total, CHUNK_SLOTS)
            ]

            for ch0, ch1 in chunks:
                cs = ch1 - ch0
                is_hier = ch0 >= s_pn
                g = gp.tile([P, CHUNK_SLOTS, D], BF16, tag="g")
                a = ap_.tile([P, CHUNK_SLOTS, D], BF16, tag="a")
                for calls, dst, idx_t in (
                    (meta["right_calls"], g, right_idx),
                    (meta["left_calls"], a, left_idx),
                ):
                    for lo, hi, bkt, kind in calls:
                        clo = max(lo, ch0 * 128)
                        chi = min(hi, ch1 * 128)
                        if clo >= chi:
                            continue
                        right_side = dst is g
                        src = (
                            wout_b[bkt]
                            if (right_side and kind != 2)
                            else win_b[bkt]
                        )
                        n = chi - clo
                        nc.gpsimd.dma_gather(
                            dst[:, (clo - ch0 * 128) // 128 : (chi - ch0 * 128) // 128, :],
                            src,
                            idx_t[:, clo // 16 : chi // 16],
                            n, n, D, single_packet=False,
                        )
                prod = prodp.tile([P, CHUNK_SLOTS, D], BF16, tag="prod")
                if is_hier and HIER_COMPUTE == "full":
                    dif = gp.tile([P, CHUNK_SLOTS, D], BF16, tag="g")
                    nc.vector.tensor_tensor(
                        out=dif[:, :cs, :], in0=a[:, :cs, :], in1=g[:, :cs, :],
                        op=mybir.AluOpType.subtract,
                    )
                    nc.scalar.activation(
                        out=prod[:, :cs, :], in_=dif[:, :cs, :],
                        func=mybir.ActivationFunctionType.Square,
                    )
                else:
                    nc.vector.tensor_tensor(
                        out=prod[:, :cs, :], in0=g[:, :cs, :], in1=a[:, :cs, :],
                        op=mybir.AluOpType.mult,
                    )
                s1 = s1p.tile([P, CHUNK_SLOTS, D // 2], BF16, tag="s1")
                nc.vector.tensor_tensor(
                    out=s1[:, :cs, :],
                    in0=prod[:, :cs, 0 : D // 2],
                    in1=prod[:, :cs, D // 2 : D],
                    op=mybir.AluOpType.add,
                )
                nc.vector.reduce_sum(
                    out=dots[:, ch0:ch1], in_=s1[:, :cs, :], axis=mybir.AxisListType.X
                )

            # S1 = sum softplus(v) over pos+neg; v = -dot (pos), +dot (neg).
            # softplus(v) = max(v,0) + ln(1+exp(-|v|)).
            eb = endp.tile([P, 2, s_pn], F32)
            nc.vector.tensor_scalar(
                out=eb[:, 0, 0:s_pos], in0=dots[:, 0:s_pos],
                scalar1=0.0, scalar2=-1.0,
                op0=mybir.AluOpType.min, op1=mybir.AluOpType.mult,
            )
            nc.vector.tensor_scalar(
                out=eb[:, 0, s_pos:s_pn], in0=dots[:, s_pos:s_pn],
                scalar1=0.0, scalar2=None, op0=mybir.AluOpType.max,
            )
            absv = endp.tile([P, s_pn], F32)
            nc.scalar.activation(
                out=absv[:], in_=dots[:, 0:s_pn],
                func=mybir.ActivationFunctionType.Abs,
            )
            expv = endp.tile([P, s_pn], F32)
            nc.scalar.activation(
                out=expv[:], in_=absv[:],
                func=mybir.ActivationFunctionType.Exp, scale=-1.0,
            )
            nc.scalar.activation(
                out=eb[:, 1, :], in_=expv[:],
                func=mybir.ActivationFunctionType.Ln, bias=1.0,
            )
            s1_acc = endp.tile([P, 1], F32)
            nc.vector.reduce_sum(out=s1_acc[:], in_=eb[:], axis=mybir.AxisListType.XY)

            # Hierarchy region: dots[:, s_pn:] already hold ||a-b||^2 per pair.
            h_acc = endp.tile([P, 1], F32)
            if s_hier > 0:
                nc.vector.reduce_sum(
                    out=h_acc[:], in_=dots[:, s_pn:s_total],
                    axis=mybir.AxisListType.X,
                )
            else:
                nc.vector.memset(h_acc[:], 0.0)

            out_sb = endp.tile([P, 2], F32)
            nc.vector.tensor_copy(out_sb[:, 0:1], s1_acc[:])
            nc.vector.tensor_copy(out_sb[:, 1:2], h_acc[:])
            nc.sync.dma_start(out_d, out_sb[:])

    nc.compile()
    return nc


def _softplus64(x):
    return float(np.logaddexp(0.0, x))


def prepare(input_labels, pos_labels, neg_labels, hierarchy_pairs,
            in_embed_w, out_embed_w):
    """Host-side: plan, program, in_maps, pad corrections."""
    w_in = np.asarray(in_embed_w, dtype=np.float32).astype(ml_dtypes.bfloat16)
    w_out = np.asarray(out_embed_w, dtype=np.float32).astype(ml_dtypes.bfloat16)

    meta, per_core = make_plan(input_labels, pos_labels, neg_labels,
                               hierarchy_pairs)
    nc = build_program(meta)

    win_slices = {
        f"win{r}": np.ascontiguousarray(w_in[r * BUCKET : (r + 1) * BUCKET])
        for r in range(NBUCKET)
    }
    wout_slices = {
        f"wout{r}": np.ascontiguousarray(w_out[r * BUCKET : (r + 1) * BUCKET])
        for r in range(NBUCKET)
    }

    # Exact pad corrections (f64 over the bf16 table values).
    w_in64 = w_in.astype(np.float64)
    w_out64 = w_out.astype(np.float64)
    pad_s = 0.0   # subtract from S (softplus sum)
    pad_h = 0.0   # subtract from H (hier sq sum)
    in_maps = []
    for k in range(NCORES):
        pc = per_core[k]
        for cnt, lb, rb, kind in pc["pad_info"]:
            lrow = w_in64[lb * BUCKET]
            if kind == 2:
                rrow = w_in64[rb * BUCKET]
                dd = (lrow - rrow).astype(ml_dtypes.bfloat16).astype(np.float64)
                pad_h += cnt * float(dd @ dd)
            else:
                rrow = w_out64[rb * BUCKET]
                d = float(lrow @ rrow)
                pad_s += cnt * _softplus64(-d if kind == 0 else d)
        in_maps.append({
            **win_slices,
            **wout_slices,
            "left_idx": pc["left_idx"],
            "right_idx": pc["right_idx"],
        })
    return nc, in_maps, (pad_s, pad_h)


def combine_results(per_core_outs, pads):
    pad_s, pad_h = pads
    s_total = 0.0
    h_total = 0.0
    for r in per_core_outs:
        o = r["out"].astype(np.float64)
        s_total += o[:, 0].sum()
        h_total += o[:, 1].sum()
    s_total -= pad_s
    h_total -= pad_h
    loss_graph = s_total / B
    loss_h = 0.5 * 1e-8 * h_total
    return (np.float32(loss_graph + loss_h), np.float32(loss_h))


def run_on_hw(nc, in_maps, **kwargs):
    from concourse.bass_utils import run_bass_kernel_spmd

    return run_bass_kernel_spmd(
        nc, in_maps, core_ids=list(range(NCORES)), **kwargs
    )


def kernel(input_labels, pos_labels, neg_labels, hierarchy_pairs,
           in_embed_w, out_embed_w):
    nc, in_maps, pads = prepare(
        input_labels, pos_labels, neg_labels, hierarchy_pairs,
        in_embed_w, out_embed_w,
    )
    res = run_on_hw(nc, in_maps)
    return combine_results(res.results, pads)
